# revision 19
# baseline (speedup 1.0000x reference)
"""Trainium2 Bass kernel for nn_AttForward (location-aware forward attention).

Math (per batch row b):
    pre   = enc_hs_pad[b] @ W_enc + b_enc                  # (T, A)
    conv  = conv1d(att_prev[b], conv_w)                    # (C, T)
    att_c = conv.T @ W_att                                 # (T, A)
    dec   = dec_z[b] @ W_dec                               # (A,)
    e     = tanh(pre + dec + att_c) @ w_g + b_g            # (T,)
    e     = mask(e); w0 = softmax(e)
    w     = normalize(clip((att_prev + shift(att_prev)) * w0, 1e-6))
    c     = enc_hs_pad[b].T @ w                            # (A_enc,)

Sharding: data-parallel over batch B=64 across 8 cores (8 rows each).

Device-side design (per core, per batch):
  - S[t, a] = pre + att_c + dec accumulates in PSUM, two 128-row t-tiles per
    [128, 1024] PSUM tile.  Per t-tile: one K=33 matmul (lhsT = [conv rows;
    ones row], rhs = [W_att; dec_b]) plus four K=128 matmuls (lhsT = encT
    tiles, rhs = W_enc tiles), all in bf16 (fp32 matmul runs as 2 HW passes
    and disables fast weight load — measured 2.4x slower).  encT is prepared
    host-side so no on-chip transpose is needed.
  - ACT applies tanh straight out of PSUM (one op per 2 t-tiles); DVE
    affine_mul_reduce fuses the multiply-by-w_g and row reduction giving e.
  - Softmax needs no max-subtraction (|e| <~ 8 analytically); the clip/renorm
    uses  normalize(clip(att*softmax(e), eps)) == normalize(max(att*exp(e),
    eps*Z))  so the softmax denominator is never divided through.
  - Cross-partition sums/broadcasts use tiny PE ones-matmuls
    (gpsimd partition_all_reduce crashes this runtime; PE versions verified).
  - c[e] = sum_t encT[e,t] w[t] via DVE affine_mul_reduce against a
    partition-broadcast copy of w (bf16), with fp32 accumulation.

b_g is dropped: softmax is shift invariant.
"""

import numpy as np
import ml_dtypes
from contextlib import ExitStack

BF16 = ml_dtypes.bfloat16

B, T, E, D, A, C, F = 64, 1024, 512, 1024, 512, 32, 15
KW = 2 * F + 1
NCORES = 8
BPC = B // NCORES  # batches per core
MASK_NEG = -50.0

_CACHE = {}


def _build_bass():
    import concourse.bass as bass
    import concourse.tile as tile
    from concourse import bacc, mybir

    f32 = mybir.dt.float32
    bf = mybir.dt.bfloat16
    nc = bacc.Bacc("TRN2", target_bir_lowering=False, debug=False, num_devices=NCORES)

    NT = T // 128  # 8 t-tiles per batch
    NK = E // 128  # 4 k-tiles of the encoder dim

    encT_d = nc.declare_dram_parameter("encT", [BPC, E, T], bf, isOutput=False)
    convp_d = nc.declare_dram_parameter("convp", [BPC, C + 1, T], bf, isOutput=False)
    catw_d = nc.declare_dram_parameter("catw", [BPC, C + 1, A], bf, isOutput=False)
    wenc_d = nc.declare_dram_parameter("wenc", [E, A], bf, isOutput=False)
    wgb_d = nc.declare_dram_parameter("wgb", [128, A], f32, isOutput=False)
    att_d = nc.declare_dram_parameter("attsum", [BPC, 128, NT], f32, isOutput=False)
    mask_d = nc.declare_dram_parameter("mask", [BPC, 128, NT], f32, isOutput=False)
    wout_d = nc.declare_dram_parameter("w_out", [BPC, 128, NT], f32, isOutput=True)
    cout_d = nc.declare_dram_parameter("c_out", [BPC, 128, NK], f32, isOutput=True)
    wtmp_d = nc.dram_tensor("wtmp", [BPC, 128, NT], bf)

    Tanh = mybir.ActivationFunctionType.Tanh
    Exp = mybir.ActivationFunctionType.Exp
    addop = mybir.AluOpType.add

    with tile.TileContext(nc) as tc, ExitStack() as ctx:
        const = ctx.enter_context(tc.tile_pool(name="const", bufs=1))
        encp = ctx.enter_context(tc.tile_pool(name="encp", bufs=2))
        convq = ctx.enter_context(tc.tile_pool(name="convq", bufs=2))
        tanhp = ctx.enter_context(tc.tile_pool(name="tanhp", bufs=3))
        smallp = ctx.enter_context(tc.tile_pool(name="smallp", bufs=4))
        junkp = ctx.enter_context(tc.tile_pool(name="junkp", bufs=1))
        wbp = ctx.enter_context(tc.tile_pool(name="wbp", bufs=2))
        psS = ctx.enter_context(tc.tile_pool(name="psS", bufs=3, space="PSUM"))
        pssm = ctx.enter_context(tc.tile_pool(name="pssm", bufs=1, space="PSUM"))

        wenc_sb = const.tile([128, NK, A], bf)
        nc.sync.dma_start(wenc_sb[:], wenc_d.ap().rearrange("(ko p) a -> p ko a", p=128))
        wg_sb = const.tile([128, A], f32)
        nc.sync.dma_start(wg_sb[:], wgb_d[:, :])
        junk = junkp.tile([128, T], f32)
        ones_c = const.tile([128, 1], f32)
        nc.vector.memset(ones_c[:], 1.0)
        ones_r = const.tile([1, 128], f32)
        nc.vector.memset(ones_r[:], 1.0)

        for b in range(BPC):
            encT_sb = encp.tile([128, NK, T], bf)
            nc.sync.dma_start(encT_sb[:], encT_d[b].rearrange("(ko p) t -> p ko t", p=128))
            convp_sb = convq.tile([C + 1, T], bf)
            nc.sync.dma_start(convp_sb[:], convp_d[b])
            catw_sb = convq.tile([C + 1, A], bf)
            nc.sync.dma_start(catw_sb[:], catw_d[b])
            att_sb = smallp.tile([128, NT], f32)
            nc.sync.dma_start(att_sb[:], att_d[b])
            mask_sb = smallp.tile([128, NT], f32)
            nc.sync.dma_start(mask_sb[:], mask_d[b])

            e_sb = smallp.tile([128, NT], f32)
            for tj in range(NT // 2):  # two t-tiles per PSUM tile
                ps = psS.tile([128, 1024], f32)
                for half in range(2):
                    ti = 2 * tj + half
                    out = ps[:, half * A : (half + 1) * A]
                    nc.tensor.matmul(
                        out, convp_sb[:, bass.ts(ti, 128)], catw_sb[:],
                        start=True, stop=False,
                    )
                    for ko in range(NK):
                        nc.tensor.matmul(
                            out, encT_sb[:, ko, bass.ts(ti, 128)], wenc_sb[:, ko, :],
                            start=False, stop=(ko == NK - 1),
                        )
                th = tanhp.tile([128, 1024], f32)
                nc.scalar.activation(th[:], ps[:], Tanh)
                for half in range(2):
                    ti = 2 * tj + half
                    nc.vector.affine_mul_reduce(
                        out=junk[:, 0:A], accum_out=e_sb[:, ti : ti + 1],
                        in0=th[:, half * A : (half + 1) * A], in1=wg_sb[:],
                        scale=1.0, bias=0.0,
                    )

            # masked softmax-free weighting:
            #   m = max(attsum * exp(e + mask), 1e-6 * Z),  w = m / sum(m)
            em = smallp.tile([128, NT], f32)
            nc.vector.tensor_add(em[:], e_sb[:], mask_sb[:])
            ex = smallp.tile([128, NT], f32)
            rs = smallp.tile([128, 1], f32)
            nc.scalar.activation(ex[:], em[:], Exp)
            nc.vector.tensor_reduce(rs[:], ex[:], axis=mybir.AxisListType.X, op=addop)
            # Z1 = cross-partition sum (PE ones-reduce); thr = 1e-6 * Z1
            # broadcast back to all partitions with a K=1 ones-matmul.
            z1ps = pssm.tile([1, 1], f32, tag="ps_scalar")
            nc.tensor.matmul(z1ps[:], ones_c[:], rs[:], start=True, stop=True)
            z1sb = smallp.tile([1, 1], f32)
            nc.scalar.mul(z1sb[:], z1ps[:], 1e-6)
            thrps = pssm.tile([128, 1], f32, tag="ps_col")
            nc.tensor.matmul(thrps[:], ones_r[:], z1sb[:], start=True, stop=True)
            thr = smallp.tile([128, 1], f32)
            nc.scalar.copy(thr[:], thrps[:])
            t1 = smallp.tile([128, NT], f32)
            nc.vector.tensor_mul(t1[:], att_sb[:], ex[:])
            mt = smallp.tile([128, NT], f32)
            nc.vector.tensor_scalar_max(mt[:], t1[:], thr[:])
            rs2 = smallp.tile([128, 1], f32)
            nc.vector.tensor_reduce(rs2[:], mt[:], axis=mybir.AxisListType.X, op=addop)
            s2ps = pssm.tile([1, 1], f32, tag="ps_scalar")
            nc.tensor.matmul(s2ps[:], ones_c[:], rs2[:], start=True, stop=True)
            s2sb = smallp.tile([1, 1], f32)
            nc.scalar.copy(s2sb[:], s2ps[:])
            rcp = smallp.tile([1, 1], f32)
            nc.vector.reciprocal(rcp[:], s2sb[:])
            rcpps = pssm.tile([128, 1], f32, tag="ps_col")
            nc.tensor.matmul(rcpps[:], ones_r[:], rcp[:], start=True, stop=True)
            rcpb = smallp.tile([128, 1], f32)
            nc.scalar.copy(rcpb[:], rcpps[:])
            wf = smallp.tile([128, NT], f32)
            nc.vector.tensor_scalar_mul(wf[:], mt[:], rcpb[:])
            nc.sync.dma_start(wout_d[b], wf[:])

            # context vector: c[e] = sum_t encT[e, t] * w[t]
            # w as a bf16 [1, T] row in t-order via DRAM round-trip
            # (t = f*128 + p), then partition-broadcast.
            wfb = smallp.tile([128, NT], bf)
            nc.vector.tensor_copy(wfb[:], wf[:])
            nc.sync.dma_start(wtmp_d[b], wfb[:])
            wrow = wbp.tile([1, T], bf)
            nc.sync.dma_start(
                wrow[0:1, :].rearrange("o (f p) -> o f p", p=128),
                wtmp_d[b].rearrange("p f -> f p"),
            )
            wb = wbp.tile([128, T], bf)
            nc.gpsimd.partition_broadcast(wb[:], wrow[:])
            ccol = smallp.tile([128, NK], f32)
            for ko in range(NK):
                nc.vector.affine_mul_reduce(
                    out=junk[:, :], accum_out=ccol[:, ko : ko + 1],
                    in0=encT_sb[:, ko, :], in1=wb[:], scale=1.0, bias=0.0,
                )
            nc.sync.dma_start(cout_d[b], ccol[:])

    nc.finalize()
    return nc


def _host_prep(enc_hs_pad, enc_hs_len, dec_z, att_prev, W_enc, b_enc, W_dec,
               W_att, conv_w, w_g, b_g):
    f = np.float32
    enc = np.asarray(enc_hs_pad, f)
    ap = np.asarray(att_prev, f)

    encT = np.ascontiguousarray(enc.transpose(0, 2, 1)).astype(BF16)       # [B, E, T]
    dec = np.asarray(dec_z, f) @ np.asarray(W_dec, f) + np.asarray(b_enc, f)  # [B, A]

    apad = np.pad(ap, ((0, 0), (F, F)))
    X = np.stack([apad[:, k : k + T] for k in range(KW)], axis=1)          # [B, KW, T]
    conv = np.einsum("ck,bkt->bct", np.asarray(conv_w, f)[:, 0, :], X)     # [B, C, T]
    convp = np.concatenate([conv, np.ones((B, 1, T), f)], axis=1)          # [B, C+1, T]

    catw = np.concatenate(
        [np.broadcast_to(np.asarray(W_att, f)[None], (B, C, A)), dec[:, None, :]],
        axis=1,
    )                                                                      # [B, C+1, A]

    attsum = ap + np.pad(ap, ((0, 0), (1, 0)))[:, :-1]                     # [B, T]
    lens = np.asarray(enc_hs_len)
    mask = np.where(np.arange(T)[None, :] >= lens[:, None], f(MASK_NEG), f(0.0))

    # interleave T as [p=128, f=T//128] (t = f*128 + p)
    def il(x):  # [B, T] -> [B, 128, T//128]
        return np.ascontiguousarray(x.reshape(B, T // 128, 128).transpose(0, 2, 1))

    return {
        "encT": encT,
        "convp": np.ascontiguousarray(convp).astype(BF16),
        "catw": np.ascontiguousarray(catw).astype(BF16),
        "wenc": np.ascontiguousarray(np.asarray(W_enc, f)).astype(BF16),
        "wgb": np.ascontiguousarray(np.broadcast_to(np.asarray(w_g, f)[None], (128, A))),
        "attsum": il(attsum.astype(f)),
        "mask": il(mask.astype(f)),
    }


def _shard(full):
    """Split host-prepped full-batch arrays into per-core in_maps."""
    in_maps = []
    for core in range(NCORES):
        sl = slice(core * BPC, (core + 1) * BPC)
        m = {}
        for k, v in full.items():
            if k in ("wenc", "wgb"):
                m[k] = v
            else:
                m[k] = np.ascontiguousarray(v[sl])
        in_maps.append(m)
    return in_maps


def _gather(results):
    w = np.concatenate([r["w_out"] for r in results], axis=0)  # [nb, 128, 8]
    c = np.concatenate([r["c_out"] for r in results], axis=0)  # [nb, 128, 4]
    nb = w.shape[0]
    w = w.transpose(0, 2, 1).reshape(nb, T)
    c = c.transpose(0, 2, 1).reshape(nb, E)
    return np.ascontiguousarray(c), np.ascontiguousarray(w)


def _install_ntff_hook():
    """The agent image's antenv lacks axon_hooks; recreate it and register
    the ctypes NTFF hook so trace=True can capture a neuron profile."""
    import sys, types
    import antenv

    if "antenv.axon_hooks" in sys.modules:
        return
    mod = types.ModuleType("antenv.axon_hooks")
    mod._hook = None
    mod.set_axon_ntff_profile_hook = lambda h: setattr(mod, "_hook", h)
    mod.get_axon_ntff_profile_hook = lambda: mod._hook
    sys.modules["antenv.axon_hooks"] = mod
    antenv.axon_hooks = mod
    try:
        from trn_agent_boot.trn_boot import _ntff_profile_via_ctypes

        mod._hook = _ntff_profile_via_ctypes("/opt/axon/libaxon_pjrt.so")
    except Exception as e:
        print("ntff hook install failed:", e)


def run(inputs, trace=False, **kw):
    if trace:
        _install_ntff_hook()
    from concourse.bass_utils import run_bass_kernel_spmd

    if "nc" not in _CACHE:
        _CACHE["nc"] = _build_bass()
    nc = _CACHE["nc"]
    in_maps = _shard(_host_prep(**inputs))
    res = run_bass_kernel_spmd(nc, in_maps, core_ids=list(range(NCORES)),
                               trace=trace, **kw)
    c, w = _gather(res.results)
    return (c, w), res


def kernel(**inputs):
    (c, w), _ = run(inputs)
    return c, w


# revision 21
# speedup vs baseline: 1.1424x; 1.1424x over previous
"""Trainium2 Bass kernel for nn_AttForward (location-aware forward attention).

Math (per batch row b):
    pre   = enc_hs_pad[b] @ W_enc + b_enc                  # (T, A)
    conv  = conv1d(att_prev[b], conv_w)                    # (C, T)
    att_c = conv.T @ W_att                                 # (T, A)
    dec   = dec_z[b] @ W_dec                               # (A,)
    e     = tanh(pre + dec + att_c) @ w_g + b_g            # (T,)
    e     = mask(e); w0 = softmax(e)
    w     = normalize(clip((att_prev + shift(att_prev)) * w0, 1e-6))
    c     = enc_hs_pad[b].T @ w                            # (A_enc,)

Sharding: data-parallel over batch B=64 across 8 cores (8 rows each).

Device-side design (per core, per batch):
  - S[t, a] = pre + att_c + dec accumulates in PSUM, two 128-row t-tiles per
    [128, 1024] PSUM tile.  Per t-tile: one K=33 matmul (lhsT = [conv rows;
    ones row], rhs = [W_att; dec_b]) plus four K=128 matmuls (lhsT = encT
    tiles, rhs = W_enc tiles), all in bf16 (fp32 matmul runs as 2 HW passes
    and disables fast weight load — measured 2.4x slower).  encT is prepared
    host-side so no on-chip transpose is needed.
  - ACT applies tanh straight out of PSUM (one op per 2 t-tiles); DVE
    affine_mul_reduce fuses the multiply-by-w_g and row reduction giving e.
  - Softmax needs no max-subtraction (|e| <~ 8 analytically); the clip/renorm
    uses  normalize(clip(att*softmax(e), eps)) == normalize(max(att*exp(e),
    eps*Z))  so the softmax denominator is never divided through.
  - Cross-partition sums/broadcasts use tiny PE ones-matmuls
    (gpsimd partition_all_reduce crashes this runtime; PE versions verified).
  - c[e] = sum_t encT[e,t] w[t] via DVE affine_mul_reduce against a
    partition-broadcast copy of w (bf16), with fp32 accumulation.

b_g is dropped: softmax is shift invariant.
"""

import numpy as np
import ml_dtypes
from contextlib import ExitStack

BF16 = ml_dtypes.bfloat16

B, T, E, D, A, C, F = 64, 1024, 512, 1024, 512, 32, 15
KW = 2 * F + 1
NCORES = 8
BPC = B // NCORES  # batches per core
MASK_NEG = -50.0

_CACHE = {}


def _build_bass():
    import concourse.bass as bass
    import concourse.tile as tile
    from concourse import bacc, mybir

    f32 = mybir.dt.float32
    bf = mybir.dt.bfloat16
    nc = bacc.Bacc("TRN2", target_bir_lowering=False, debug=False, num_devices=NCORES)

    NT = T // 128  # 8 t-tiles per batch
    NK = E // 128  # 4 k-tiles of the encoder dim

    encT_d = nc.declare_dram_parameter("encT", [BPC, E, T], bf, isOutput=False)
    convp_d = nc.declare_dram_parameter("convp", [BPC, C + 1, T], bf, isOutput=False)
    catw_d = nc.declare_dram_parameter("catw", [BPC, C + 1, A], bf, isOutput=False)
    wenc_d = nc.declare_dram_parameter("wenc", [E, A], bf, isOutput=False)
    wgb_d = nc.declare_dram_parameter("wgb", [128, A], bf, isOutput=False)
    att_d = nc.declare_dram_parameter("attsum", [BPC, 128, NT], f32, isOutput=False)
    mask_d = nc.declare_dram_parameter("mask", [BPC, 128, NT], f32, isOutput=False)
    wout_d = nc.declare_dram_parameter("w_out", [BPC, 128, NT], f32, isOutput=True)
    cout_d = nc.declare_dram_parameter("c_out", [BPC, 128, NK], f32, isOutput=True)
    wtmp_d = nc.dram_tensor("wtmp", [BPC, 128, NT], bf)

    Tanh = mybir.ActivationFunctionType.Tanh
    Exp = mybir.ActivationFunctionType.Exp
    addop = mybir.AluOpType.add

    with tile.TileContext(nc) as tc, ExitStack() as ctx:
        const = ctx.enter_context(tc.tile_pool(name="const", bufs=1))
        encp = ctx.enter_context(tc.tile_pool(name="encp", bufs=3))
        convq = ctx.enter_context(tc.tile_pool(name="convq", bufs=3))
        tanhp = ctx.enter_context(tc.tile_pool(name="tanhp", bufs=3))
        smallp = ctx.enter_context(tc.tile_pool(name="smallp", bufs=4))
        junkp = ctx.enter_context(tc.tile_pool(name="junkp", bufs=1))
        wbp = ctx.enter_context(tc.tile_pool(name="wbp", bufs=2))
        psS = ctx.enter_context(tc.tile_pool(name="psS", bufs=3, space="PSUM"))

        wenc_sb = const.tile([128, NK, A], bf)
        nc.sync.dma_start(wenc_sb[:], wenc_d.ap().rearrange("(ko p) a -> p ko a", p=128))
        wg_sb = const.tile([128, A], bf)
        nc.sync.dma_start(wg_sb[:], wgb_d[:, :])
        junk = junkp.tile([128, T], f32)

        for b in range(BPC):
            encT_sb = encp.tile([128, NK, T], bf)
            nc.sync.dma_start(encT_sb[:], encT_d[b].rearrange("(ko p) t -> p ko t", p=128))
            convp_sb = convq.tile([C + 1, T], bf)
            nc.sync.dma_start(convp_sb[:], convp_d[b])
            catw_sb = convq.tile([C + 1, A], bf)
            nc.sync.dma_start(catw_sb[:], catw_d[b])
            att_sb = smallp.tile([128, NT], f32)
            nc.sync.dma_start(att_sb[:], att_d[b])
            mask_sb = smallp.tile([128, NT], f32)
            nc.sync.dma_start(mask_sb[:], mask_d[b])

            e_sb = smallp.tile([128, NT], f32)
            for tj in range(NT // 2):  # two t-tiles per PSUM tile
                ps = psS.tile([128, 1024], f32)
                for half in range(2):
                    ti = 2 * tj + half
                    out = ps[:, half * A : (half + 1) * A]
                    nc.tensor.matmul(
                        out, convp_sb[:, bass.ts(ti, 128)], catw_sb[:],
                        start=True, stop=False,
                    )
                    for ko in range(NK):
                        nc.tensor.matmul(
                            out, encT_sb[:, ko, bass.ts(ti, 128)], wenc_sb[:, ko, :],
                            start=False, stop=(ko == NK - 1),
                        )
                th = tanhp.tile([128, 1024], bf)
                nc.scalar.activation(th[:], ps[:], Tanh)
                for half in range(2):
                    ti = 2 * tj + half
                    nc.vector.affine_mul_reduce(
                        out=junk[:, 0:A], accum_out=e_sb[:, ti : ti + 1],
                        in0=th[:, half * A : (half + 1) * A], in1=wg_sb[:],
                        scale=1.0, bias=0.0,
                    )

            # masked softmax-free weighting:
            #   m = max(attsum * exp(e + mask), 1e-6 * Z),  w = m / sum(m)
            em = smallp.tile([128, NT], f32)
            nc.vector.tensor_add(em[:], e_sb[:], mask_sb[:])
            ex = smallp.tile([128, NT], f32)
            rs = smallp.tile([128, 1], f32)
            nc.scalar.activation(ex[:], em[:], Exp)
            nc.vector.tensor_reduce(rs[:], ex[:], axis=mybir.AxisListType.X, op=addop)
            # Z1 = cross-partition sum: relayout [128,1] -> [1,128] with a
            # tiny SBUF->SBUF DMA, row-reduce on DVE, broadcast on GPSIMD.
            # (Keeps the PE instruction stream free of softmax dependencies.)
            rsr = smallp.tile([1, 128], f32)
            nc.sync.dma_start(rsr[0:1, :].rearrange("o (x p) -> o x p", p=128), rs[:])
            z1sb = smallp.tile([1, 1], f32)
            nc.vector.tensor_reduce(z1sb[:], rsr[:], axis=mybir.AxisListType.X, op=addop)
            thr1 = smallp.tile([1, 1], f32)
            nc.scalar.mul(thr1[:], z1sb[:], 1e-6)
            thr = smallp.tile([128, 1], f32)
            nc.gpsimd.partition_broadcast(thr[:], thr1[:])
            t1 = smallp.tile([128, NT], f32)
            nc.vector.tensor_mul(t1[:], att_sb[:], ex[:])
            mt = smallp.tile([128, NT], f32)
            nc.vector.tensor_scalar_max(mt[:], t1[:], thr[:])
            rs2 = smallp.tile([128, 1], f32)
            nc.vector.tensor_reduce(rs2[:], mt[:], axis=mybir.AxisListType.X, op=addop)
            rsr2 = smallp.tile([1, 128], f32)
            nc.sync.dma_start(rsr2[0:1, :].rearrange("o (x p) -> o x p", p=128), rs2[:])
            s2sb = smallp.tile([1, 1], f32)
            nc.vector.tensor_reduce(s2sb[:], rsr2[:], axis=mybir.AxisListType.X, op=addop)
            rcp = smallp.tile([1, 1], f32)
            nc.vector.reciprocal(rcp[:], s2sb[:])
            rcpb = smallp.tile([128, 1], f32)
            nc.gpsimd.partition_broadcast(rcpb[:], rcp[:])
            wf = smallp.tile([128, NT], f32)
            nc.vector.tensor_scalar_mul(wf[:], mt[:], rcpb[:])
            nc.sync.dma_start(wout_d[b], wf[:])

            # context vector: c[e] = sum_t encT[e, t] * w[t]
            # w as a bf16 [1, T] row in t-order via DRAM round-trip
            # (t = f*128 + p), then partition-broadcast.
            wfb = smallp.tile([128, NT], bf)
            nc.vector.tensor_copy(wfb[:], wf[:])
            nc.sync.dma_start(wtmp_d[b], wfb[:])
            wrow = wbp.tile([1, T], bf)
            nc.sync.dma_start(
                wrow[0:1, :].rearrange("o (f p) -> o f p", p=128),
                wtmp_d[b].rearrange("p f -> f p"),
            )
            wb = wbp.tile([128, T], bf)
            nc.gpsimd.partition_broadcast(wb[:], wrow[:])
            ccol = smallp.tile([128, NK], f32)
            for ko in range(NK):
                nc.vector.affine_mul_reduce(
                    out=junk[:, :], accum_out=ccol[:, ko : ko + 1],
                    in0=encT_sb[:, ko, :], in1=wb[:], scale=1.0, bias=0.0,
                )
            nc.sync.dma_start(cout_d[b], ccol[:])

    nc.finalize()
    return nc


def _host_prep(enc_hs_pad, enc_hs_len, dec_z, att_prev, W_enc, b_enc, W_dec,
               W_att, conv_w, w_g, b_g):
    f = np.float32
    enc = np.asarray(enc_hs_pad, f)
    ap = np.asarray(att_prev, f)

    encT = np.ascontiguousarray(enc.transpose(0, 2, 1)).astype(BF16)       # [B, E, T]
    dec = np.asarray(dec_z, f) @ np.asarray(W_dec, f) + np.asarray(b_enc, f)  # [B, A]

    apad = np.pad(ap, ((0, 0), (F, F)))
    X = np.stack([apad[:, k : k + T] for k in range(KW)], axis=1)          # [B, KW, T]
    conv = np.einsum("ck,bkt->bct", np.asarray(conv_w, f)[:, 0, :], X)     # [B, C, T]
    convp = np.concatenate([conv, np.ones((B, 1, T), f)], axis=1)          # [B, C+1, T]

    catw = np.concatenate(
        [np.broadcast_to(np.asarray(W_att, f)[None], (B, C, A)), dec[:, None, :]],
        axis=1,
    )                                                                      # [B, C+1, A]

    attsum = ap + np.pad(ap, ((0, 0), (1, 0)))[:, :-1]                     # [B, T]
    lens = np.asarray(enc_hs_len)
    mask = np.where(np.arange(T)[None, :] >= lens[:, None], f(MASK_NEG), f(0.0))

    # interleave T as [p=128, f=T//128] (t = f*128 + p)
    def il(x):  # [B, T] -> [B, 128, T//128]
        return np.ascontiguousarray(x.reshape(B, T // 128, 128).transpose(0, 2, 1))

    return {
        "encT": encT,
        "convp": np.ascontiguousarray(convp).astype(BF16),
        "catw": np.ascontiguousarray(catw).astype(BF16),
        "wenc": np.ascontiguousarray(np.asarray(W_enc, f)).astype(BF16),
        "wgb": np.ascontiguousarray(np.broadcast_to(np.asarray(w_g, f)[None], (128, A))).astype(BF16),
        "attsum": il(attsum.astype(f)),
        "mask": il(mask.astype(f)),
    }


def _shard(full):
    """Split host-prepped full-batch arrays into per-core in_maps."""
    in_maps = []
    for core in range(NCORES):
        sl = slice(core * BPC, (core + 1) * BPC)
        m = {}
        for k, v in full.items():
            if k in ("wenc", "wgb"):
                m[k] = v
            else:
                m[k] = np.ascontiguousarray(v[sl])
        in_maps.append(m)
    return in_maps


def _gather(results):
    w = np.concatenate([r["w_out"] for r in results], axis=0)  # [nb, 128, 8]
    c = np.concatenate([r["c_out"] for r in results], axis=0)  # [nb, 128, 4]
    nb = w.shape[0]
    w = w.transpose(0, 2, 1).reshape(nb, T)
    c = c.transpose(0, 2, 1).reshape(nb, E)
    return np.ascontiguousarray(c), np.ascontiguousarray(w)


def _install_ntff_hook():
    """The agent image's antenv lacks axon_hooks; recreate it and register
    the ctypes NTFF hook so trace=True can capture a neuron profile."""
    import sys, types
    import antenv

    if "antenv.axon_hooks" in sys.modules:
        return
    mod = types.ModuleType("antenv.axon_hooks")
    mod._hook = None
    mod.set_axon_ntff_profile_hook = lambda h: setattr(mod, "_hook", h)
    mod.get_axon_ntff_profile_hook = lambda: mod._hook
    sys.modules["antenv.axon_hooks"] = mod
    antenv.axon_hooks = mod
    try:
        from trn_agent_boot.trn_boot import _ntff_profile_via_ctypes

        mod._hook = _ntff_profile_via_ctypes("/opt/axon/libaxon_pjrt.so")
    except Exception as e:
        print("ntff hook install failed:", e)


def run(inputs, trace=False, **kw):
    if trace:
        _install_ntff_hook()
    from concourse.bass_utils import run_bass_kernel_spmd

    if "nc" not in _CACHE:
        _CACHE["nc"] = _build_bass()
    nc = _CACHE["nc"]
    in_maps = _shard(_host_prep(**inputs))
    res = run_bass_kernel_spmd(nc, in_maps, core_ids=list(range(NCORES)),
                               trace=trace, **kw)
    c, w = _gather(res.results)
    return (c, w), res


def kernel(**inputs):
    (c, w), _ = run(inputs)
    return c, w
